# revision 31
# baseline (speedup 1.0000x reference)
"""DiffAttention Trainium2 kernel (8-core SPMD), fp16 matmul pipeline.

Problem shapes: b=4, t=1024, d=1024, H=16 v-heads (2H=32 q/k heads), E=64.
Sharding: batch x head-block. Core c handles batch c//2 and v-heads
[8*(c%2), 8*(c%2)+8)  (= q/k heads [16*(c%2), 16*(c%2)+16)).

Per-core device program (matmuls in fp16, accumulation in fp32 PSUM):
  pair-0 QK projection first, then per pair p: attention over 2
  q-blocks x 8 k-chunks with the V projection (pair 0, q-block 0) and
  the QK projection matmuls of pair p+1 interleaved into the attention
  loop so the PE fills ScalarE (exp) wait bubbles with projection work.

  Attention inner loop per (pair, q-block, k-chunk):
    s[128k, 2, 512q] PSUM <- pos head scores (PE rows 0-63) and neg head
      scores (PE rows 64-127), issued back-to-back so the two row-group
      matmuls stream concurrently on the PE array;
    e = exp(s) on ScalarE -> fp16 (bias -3, cancels in normalization);
    o[128, 512] PSUM: o[0:64] += V^T e_pos on PE column tile (0,0) and
      o[64:128] += V^T e_neg on column tile (0,64) -- the two matmuls
      share the V stationary and stream concurrently;
    P[128, 2, 512] SBUF fp16 += e on VectorE (denominator partials).
  o and P are DMA'd raw to DRAM (o via a VectorE fp32->fp16 copy).

Host side: shard + cast inputs to fp16, fold the E^-0.25 scales into
Wq/Wk; after gather, den = P.sum(keys), out = o_pos/den_pos -
lam*o_neg/den_neg, transpose to (b, t, H*E).
"""

import numpy as np
from contextlib import ExitStack

import concourse.bass as bass
import concourse.tile as tile
from concourse import bacc, mybir
from concourse.bass_utils import run_bass_kernel_spmd

F32 = mybir.dt.float32
F16 = mybir.dt.float16
EXP = mybir.ActivationFunctionType.Exp

E = 64          # per-head embed
H = 16          # global v-heads
B = 4           # batch
T = 1024        # sequence length
D = 1024        # model dim
N_CORES = 8
LAMBDA_INIT = 0.2
EXP_BIAS = -3.0  # cancels in normalization; keeps fp16 e comfortably ranged

# per-core sizes
NQKH = 16                  # local q/k heads
PAIRS = NQKH // 2          # local head pairs / v heads
HE = NQKH * E              # 1024, q/k projection width
VHE = PAIRS * E            # 512, v projection width / output width
DC = D // 128              # contraction chunks
KC = T // 128              # key-position chunks
QB = T // 512              # query blocks of 512
NG = PAIRS * QB            # output blocks per core


def build_bass(mm_dt=F16):
    nc = bacc.Bacc("TRN2", target_bir_lowering=False, debug=False,
                   num_devices=N_CORES)

    xqT = nc.dram_tensor("xqT", [D, T], mm_dt, kind="ExternalInput").ap()
    xkT = nc.dram_tensor("xkT", [D, T], mm_dt, kind="ExternalInput").ap()
    xvT = nc.dram_tensor("xvT", [D, T], mm_dt, kind="ExternalInput").ap()
    wqT = nc.dram_tensor("wqT", [D, HE], mm_dt, kind="ExternalInput").ap()
    wkT = nc.dram_tensor("wkT", [D, HE], mm_dt, kind="ExternalInput").ap()
    wvT = nc.dram_tensor("wvT", [D, VHE], mm_dt, kind="ExternalInput").ap()
    # numerators: [block, e, pos/neg stacked on partitions, q]
    out = nc.dram_tensor("out", [NG, 128, 512], F16,
                         kind="ExternalOutput").ap()
    # denominator partials: [block, key-chunk partition, pos/neg, q]
    pden = nc.dram_tensor("pden", [NG, 128, 2, 512], F16,
                          kind="ExternalOutput").ap()

    mm = nc.tensor.matmul

    with tile.TileContext(nc) as tc, ExitStack() as ctx:
        res = ctx.enter_context(tc.tile_pool(name="res", bufs=1))
        pin = ctx.enter_context(tc.tile_pool(name="pin", bufs=1))
        ppsum = ctx.enter_context(tc.tile_pool(name="ppsum", bufs=2,
                                               space="PSUM"))
        s_pool = ctx.enter_context(tc.tile_pool(name="s", bufs=2,
                                                space="PSUM"))
        o_pool = ctx.enter_context(tc.tile_pool(name="o", bufs=2,
                                                space="PSUM"))
        pexp_pool = ctx.enter_context(tc.tile_pool(name="pexp", bufs=4))
        pacc_pool = ctx.enter_context(tc.tile_pool(name="pacc", bufs=2))
        post_pool = ctx.enter_context(tc.tile_pool(name="post", bufs=3))

        QT = [res.tile([128, T], mm_dt, tag=f"QT{i}", name=f"QT{i}")
              for i in range(PAIRS)]
        KT = [res.tile([128, T], mm_dt, tag=f"KT{i}", name=f"KT{i}")
              for i in range(PAIRS)]
        VB = [res.tile([128, PAIRS, E], mm_dt, tag=f"VB{i}", name=f"VB{i}")
              for i in range(KC)]
        ebias = res.tile([128, 1], F32, tag="ebias", name="ebias")
        nc.vector.memset(ebias, EXP_BIAS)
        scr_a = res.tile([128, 128], mm_dt, tag="scra", name="scr_a")
        scr_b = res.tile([128, 512], mm_dt, tag="scrb", name="scr_b")
        nc.vector.memset(scr_a, 0.0)
        nc.vector.memset(scr_b, 0.0)

        # resident inputs (fp16): x for q/k/v and full weight panels
        xv_sb = [pin.tile([128, T], mm_dt, tag=f"xv{i}", name=f"xv{i}")
                 for i in range(DC)]
        wv_sb = [pin.tile([128, VHE], mm_dt, tag=f"wv{i}", name=f"wv{i}")
                 for i in range(DC)]
        xq_sb = [pin.tile([128, T], mm_dt, tag=f"xq{i}", name=f"xq{i}")
                 for i in range(DC)]
        wq_sb = [pin.tile([128, HE], mm_dt, tag=f"wq{i}", name=f"wq{i}")
                 for i in range(DC)]
        xk_sb = [pin.tile([128, T], mm_dt, tag=f"xk{i}", name=f"xk{i}")
                 for i in range(DC)]
        wk_sb = [pin.tile([128, HE], mm_dt, tag=f"wk{i}", name=f"wk{i}")
                 for i in range(DC)]
        # DMA order matches consumption (V inputs first, pair-0 weight
        # slices ahead of the rest); x tensors issue from the sync queue
        # while weight panels issue in parallel from the gpsimd queue.
        for i in range(DC):
            nc.sync.dma_start(out=xv_sb[i], in_=xvT[i * 128:(i + 1) * 128, :])
            nc.gpsimd.dma_start(out=wv_sb[i],
                                in_=wvT[i * 128:(i + 1) * 128, :])
        for i in range(DC):
            nc.sync.dma_start(out=xk_sb[i], in_=xkT[i * 128:(i + 1) * 128, :])
            nc.gpsimd.dma_start(out=wk_sb[i][:, 0:128],
                                in_=wkT[i * 128:(i + 1) * 128, 0:128])
        for i in range(DC):
            nc.sync.dma_start(out=xq_sb[i], in_=xqT[i * 128:(i + 1) * 128, :])
            nc.gpsimd.dma_start(out=wq_sb[i][:, 0:128],
                                in_=wqT[i * 128:(i + 1) * 128, 0:128])
        for i in range(DC):
            nc.gpsimd.dma_start(out=wk_sb[i][:, 128:HE],
                                in_=wkT[i * 128:(i + 1) * 128, 128:HE])
        for i in range(DC):
            nc.gpsimd.dma_start(out=wq_sb[i][:, 128:HE],
                                in_=wqT[i * 128:(i + 1) * 128, 128:HE])

        # V projection for one 128-key chunk
        def emit_v_chunk(tcn):
            ps = ppsum.tile([128, 512], F32, tag="ps", name="psv")
            for dc in range(DC):
                mm(ps, xv_sb[dc][:, tcn * 128:(tcn + 1) * 128],
                   wv_sb[dc],
                   start=(dc == 0), stop=(dc == DC - 1))
            nc.vector.tensor_copy(VB[tcn],
                                  ps.rearrange("p (h e) -> p h e", e=E))

        # QK projection matmuls for one pair (k first: scores of block
        # (p, qb0) need all of KT[p] but only the tq0 half of QT[p]).
        def qk_proj_ops(p):
            ops = []
            csl = slice(p * 128, (p + 1) * 128)
            for (x_sb, w_sb, OUT) in ((xk_sb, wk_sb, KT), (xq_sb, wq_sb, QT)):
                for tq in range(T // 512):
                    ps = ppsum.tile([128, 512], F32, tag="ps", name="psqk")
                    for dc in range(DC):
                        ops.append((ps, w_sb[dc], csl, x_sb[dc], tq, dc,
                                    OUT[p]))
            return ops

        def emit_proj(op):
            ps, w, csl, x, tq, dc, dst = op
            mm(ps, w[:, csl], x[:, tq * 512:(tq + 1) * 512],
               start=(dc == 0), stop=(dc == DC - 1))
            if dc == DC - 1:
                nc.vector.tensor_copy(dst[:, tq * 512:(tq + 1) * 512], ps)

        # Dummy matmuls on zeroed scratch while the input DMAs are in
        # flight: ~3.4us of sustained PE activity trips the HAM clock
        # gate to 8/8, and keeping bursts between the DMA-paced real
        # matmuls prevents it from re-throttling, so the prologue runs
        # at the warm rate.
        def emit_warm(n):
            for w in range(n):
                sd = s_pool.tile([128, 2, 512], F32, tag="s", name="warm")
                mm(sd[:, 0, :], scr_a, scr_b, start=True, stop=True)

        emit_warm(12)

        # prologue: V projection (inputs arrive first), then pair-0 QK
        for tcn in range(KC):
            emit_v_chunk(tcn)
        for op in qk_proj_ops(0):
            emit_proj(op)

        # Global queue of pair-1..7 projection ops, interleaved into the
        # attention loop. Per-block quotas shift 8 ops into pair 7 (its
        # QT tq1 half, needed only at (7, qb1)) so the PE stays dense at
        # the tail; force-drain before each block covers any remainder.
        queue = []
        for p in range(1, PAIRS):
            queue += qk_proj_ops(p)
        qi = 0

        QUOTA = [2] * 13 + [1, 1, 0]

        def proj_quota(p, qb):
            return QUOTA[p * QB + qb]

        # ---------------- attention, pipelined ----------------
        for p in range(PAIRS):
            for qb in range(QB):
                # prerequisites: KT[p] full + QT[p] tq half for this block
                need = 0 if p == 0 else 32 * (p - 1) + (24 if qb == 0 else 32)
                while qi < need:
                    emit_proj(queue[qi])
                    qi += 1
                qsl = slice(qb * 512, (qb + 1) * 512)
                nproj = proj_quota(p, qb)
                o = o_pool.tile([128, 512], F32, tag="o", name="o")
                P = pacc_pool.tile([128, 2, 512], mm_dt, tag="P", name="P")
                e_tiles = [None] * KC
                for kc in range(KC):
                    ksl = slice(kc * 128, (kc + 1) * 128)
                    s = s_pool.tile([128, 2, 512], F32, tag="s", name="s")
                    mm(s[:, 0, :], KT[p][0:64, ksl], QT[p][0:64, qsl],
                       start=True, stop=True, tile_position=(0, 0))
                    mm(s[:, 1, :], KT[p][64:128, ksl], QT[p][64:128, qsl],
                       start=True, stop=True, tile_position=(64, 0))
                    e = pexp_pool.tile([128, 2, 512], mm_dt, tag="e", name="e")
                    nc.scalar.activation(e, s, EXP, bias=ebias)
                    e_tiles[kc] = e
                    # fill the exp-wait bubble with projection work
                    for _ in range(nproj):
                        if qi < len(queue):
                            emit_proj(queue[qi])
                            qi += 1
                    # attn@V one k-chunk behind; pos/neg on concurrent
                    # column tiles sharing the V stationary
                    if kc > 0:
                        mm(o[0:64, :], VB[kc - 1][:, p, :],
                           e_tiles[kc - 1][:, 0, :],
                           start=(kc - 1 == 0), stop=False,
                           tile_position=(0, 0))
                        mm(o[64:128, :], VB[kc - 1][:, p, :],
                           e_tiles[kc - 1][:, 1, :],
                           start=(kc - 1 == 0), stop=False,
                           tile_position=(0, 64))
                    # denominator partials on VectorE (fp16 2x mode)
                    if kc == 1:
                        nc.vector.tensor_add(P, e_tiles[0], e_tiles[1])
                    elif kc > 1:
                        nc.vector.tensor_add(P, P, e_tiles[kc])
                mm(o[0:64, :], VB[KC - 1][:, p, :], e_tiles[KC - 1][:, 0, :],
                   start=False, stop=True, tile_position=(0, 0))
                mm(o[64:128, :], VB[KC - 1][:, p, :], e_tiles[KC - 1][:, 1, :],
                   start=False, stop=True, tile_position=(0, 64))

                osb = post_pool.tile([128, 512], F16, tag="osb", name="osb")
                nc.vector.tensor_copy(osb, o)
                # split output DMAs so the transfers run on parallel DMA
                # engines (a single 256KB transfer is ~13us -- it would
                # dominate the kernel tail)
                g = p * QB + qb
                nc.sync.dma_start(out=out[g][:, 0:256], in_=osb[:, 0:256])
                nc.sync.dma_start(out=out[g][:, 256:512],
                                  in_=osb[:, 256:512])
                oq = [nc.sync, nc.gpsimd, nc.sync, nc.gpsimd]
                for h in range(4):
                    cs = slice(h * 128, (h + 1) * 128)
                    oq[h].dma_start(out=pden[g][:, :, cs], in_=P[:, :, cs])
        while qi < len(queue):
            emit_proj(queue[qi])
            qi += 1

    nc.compile()
    return nc


def make_in_maps(q_input, k_input, v_input, Wq, Wk, Wv):
    scale = np.float32(E ** -0.25)
    in_maps = []
    for c in range(N_CORES):
        b, hb = c // 2, c % 2
        in_maps.append({
            "xqT": np.ascontiguousarray(q_input[b].T).astype(np.float16),
            "xkT": np.ascontiguousarray(k_input[b].T).astype(np.float16),
            "xvT": np.ascontiguousarray(v_input[b].T).astype(np.float16),
            "wqT": (np.ascontiguousarray(Wq[1024 * hb:1024 * (hb + 1), :].T)
                    * scale).astype(np.float16),
            "wkT": (np.ascontiguousarray(Wk[1024 * hb:1024 * (hb + 1), :].T)
                    * scale).astype(np.float16),
            "wvT": np.ascontiguousarray(
                Wv[512 * hb:512 * (hb + 1), :].T).astype(np.float16),
        })
    return in_maps


_NC_CACHE = {}


def get_nc(mm_dt=F16):
    key = str(mm_dt)
    if key not in _NC_CACHE:
        _NC_CACHE[key] = build_bass(mm_dt)
    return _NC_CACHE[key]


def kernel(q_input, k_input, v_input, Wq, Wk, Wv, L, _trace=False):
    q_input = np.asarray(q_input, np.float32)
    k_input = np.asarray(k_input, np.float32)
    v_input = np.asarray(v_input, np.float32)
    Wq = np.asarray(Wq, np.float32)
    Wk = np.asarray(Wk, np.float32)
    Wv = np.asarray(Wv, np.float32)
    L = np.asarray(L, np.float32)

    lam = np.float32(LAMBDA_INIT + np.exp(np.float32(L[0] @ L[1]))
                     - np.exp(np.float32(L[2] @ L[3])))

    nc = get_nc()
    in_maps = make_in_maps(q_input, k_input, v_input, Wq, Wk, Wv)
    res = run_bass_kernel_spmd(nc, in_maps, list(range(N_CORES)), trace=_trace)

    full = np.empty((B, T, H * E), np.float32)
    for c in range(N_CORES):
        b, hb = c // 2, c % 2
        raw = np.asarray(res.results[c]["out"], np.float32)   # [NG,128,512]
        P = np.asarray(res.results[c]["pden"], np.float32)    # [NG,128,2,512]
        den = P.sum(axis=1)                                   # [NG,2,512]
        z = (raw[:, 0:E, :] / den[:, None, 0, :]
             - lam * raw[:, E:2 * E, :] / den[:, None, 1, :])  # [NG,64,512]
        # block g = p*QB+qb holds queries [qb*512,(qb+1)*512), head pair p
        zz = (z.reshape(PAIRS, QB, E, 512)
               .transpose(1, 3, 0, 2)                         # [QB,512,PAIRS,E]
               .reshape(T, VHE))
        full[b, :, VHE * hb:VHE * (hb + 1)] = zz
    if _trace:
        return full, res
    return full
